# revision 17
# baseline (speedup 1.0000x reference)
"""Trainium2 Bass kernel for nn_Net_5488968204310 (gnn_message_passing).

Single-head self-attention (D=128) over N=1024 nodes + gated residual update,
batch B=32, data-parallel across 8 NeuronCores (4 samples per core).

Design notes:
  - "T layout" (features d on partitions, nodes on free dim) for every matmul;
    contraction is always over d.
  - QK fold: logit_ij = x_i.(M^T x_j) with M = Wq@Wk^T folded on the host, so
    only ONE projection matmul (yT = (Wk Wq^T)^T... stationary S=Wk@Wq^T) is
    needed and the logits' moving operand is xT itself. The per-query bias
    term is softmax-invariant and dropped; the per-key term folds into y's
    bias (Wk@bq).
  - V fold: attn@Wo = (A@X)@(Wv@Wo). Attention runs over raw X (host-cast
    fp8 upload, natural layout) and Wv folds into the tail weights, removing
    the v projection and its PSUM->SBUF cast.
  - QK^T: logitsT chunks [m_chunk(128) x q(1024)] = yT_chunk.T @ xT; exp() on
    the scalar engine straight from PSUM with the 1/sqrt(D) scale and a -2
    bias folded in (rescales exp into fp8 range; the uniform e^-2 factor
    cancels between numerator and denominator).
  - A@X keeps xf8 as the stationary operand (DoubleRow fp8, dense 512-col
    streams). The softmax denominator is a parallel ones.T @ expw DoubleRow
    accumulation; 1/denom via the DVE reciprocal_approx_fast custom op.
  - gate sigmoid as 0.5*(1+tanh(z/2)): one ACT op (Tanh) instead of a 3-op
    exp/ln chain. The 0.5 gate factor is folded into the u-path weights on
    the host so dlt = u_half * (tanh+1) is a single DVE scalar_tensor_tensor.
  - ACT functions used (Exp, Tanh, Identity/Relu fallbacks) all live in the
    exp_and_others table set, enforced by a scoped patch of the table
    metadata at compile time: exactly one ACT_TABLE_LOAD per run.
  - The residual add runs in bf16 on the Pool engine (x loaded bf16); all
    six folded 128x128 weights ship as ONE packed DMA to cut issue latency.
"""

import math

import numpy as np
import ml_dtypes

B, N, D = 32, 1024, 128
NCORES = 8
BPC = B // NCORES  # samples per core
NT = N // 128      # node chunks per sample

WNAMES = ["Wm", "Woh", "Wo1mh", "Wg1", "Wog2", "Wg3"]

_CACHE = {}


def _bias_mode(vec):
    """(kind, value) where kind in {'zero', 'uniform', 'ap'}."""
    v = np.asarray(vec, np.float32)
    if not np.any(v):
        return ("zero", 0.0)
    if np.all(v == v.flat[0]):
        return ("uniform", float(v.flat[0]))
    return ("ap", 0.0)


def _build_nc(modes):
    import concourse.bacc as bacc
    import concourse.tile as tile
    from concourse import mybir
    from contextlib import ExitStack

    f32 = mybir.dt.float32
    bf16 = mybir.dt.bfloat16
    f8 = mybir.dt.float8e4
    AF = mybir.ActivationFunctionType
    OP = mybir.AluOpType

    nc = bacc.Bacc("TRN2", target_bir_lowering=False, debug=False)

    xb_d = nc.dram_tensor("xbf", [BPC, N, D], bf16, kind="ExternalInput")
    xt_d = nc.dram_tensor("xtb", [BPC, D, N], bf16, kind="ExternalInput")
    xf8_d = nc.dram_tensor("xf8", [BPC, N, D], f8, kind="ExternalInput")
    out_d = nc.dram_tensor("out", [BPC, N, D], f32, kind="ExternalOutput")
    wp_d = nc.dram_tensor("wpack", [D, len(WNAMES), D], bf16, kind="ExternalInput")
    b_d = {
        n: nc.dram_tensor(n, [D, 1], f32, kind="ExternalInput")
        for n in modes if modes[n][0] == "ap"
    }

    s = 1.0 / math.sqrt(D)

    with tile.TileContext(nc) as tc, ExitStack() as ctx:
        consts = ctx.enter_context(tc.tile_pool(name="consts", bufs=1))
        sb = ctx.enter_context(tc.tile_pool(name="sb", bufs=2))
        sb3 = ctx.enter_context(tc.tile_pool(name="sb3", bufs=3))
        expp = ctx.enter_context(tc.tile_pool(name="expp", bufs=2))
        pw = ctx.enter_context(tc.tile_pool(name="pw", bufs=2, space="PSUM"))
        ph = ctx.enter_context(tc.tile_pool(name="ph", bufs=2, space="PSUM"))
        pav = ctx.enter_context(tc.tile_pool(name="pav", bufs=1, space="PSUM"))
        pden = ctx.enter_context(tc.tile_pool(name="pden", bufs=1, space="PSUM"))

        ST = {}

        def load(b):
            """input DMAs for sample b (issued one pipeline step early)."""
            st = {}
            xT = sb3.tile([128, N], bf16, tag="xT")  # [d, n]
            nc.sync.dma_start(xT, xt_d[b])
            x_nat = sb3.tile([128, NT, D], bf16, tag="x_nat")
            nc.sync.dma_start(x_nat, xb_d[b].rearrange("(c p) d -> p c d", p=128))
            xf8_nat = sb3.tile([128, NT, D], f8, tag="xf8_nat")
            nc.sync.dma_start(xf8_nat, xf8_d[b].rearrange("(c p) d -> p c d", p=128))
            st["x_nat"], st["xf8_nat"], st["xT"] = x_nat, xf8_nat, xT
            return st

        # input DMAs for sample 0 go out before anything else; the packed
        # weight DMA + consts follow on other engines so nothing serializes
        # behind the sequencer's per-DMA issue cost.
        ST[0] = load(0)

        wpack = consts.tile([D, len(WNAMES), D], bf16, tag="wpack")
        nc.scalar.dma_start(wpack, wp_d[:, :, :])
        W = {n: wpack[:, i, :] for i, n in enumerate(WNAMES)}
        ones_dr = consts.tile([128, 2, 128], f8, tag="ones_dr")
        nc.gpsimd.memset(ones_dr, 1.0)
        expbias = consts.tile([128, 1], f32, tag="expbias")
        nc.gpsimd.memset(expbias, -2.0)
        BV = {}
        for n in b_d:
            t = consts.tile([D, 1], f32, tag=f"b_{n}")
            nc.scalar.dma_start(t, b_d[n][:, :])
            BV[n] = t
        for n, (kind, val) in modes.items():
            if kind == "uniform":
                t = consts.tile([D, 1], f32, tag=f"b_{n}")
                nc.gpsimd.memset(t, val)
                BV[n] = t

        # PE p-state warmup: ~16 dummy DoubleRow matmuls on the ones const
        # while the first x DMAs are still in flight, so the first real
        # matmuls run at full clock instead of the cold 0.65 GHz p-state.
        pdum = pden.tile([128, 512], f32, tag="pden", name="pdum")
        for _ in range(16):
            nc.tensor.matmul(
                pdum[:, 0:128], ones_dr, ones_dr, start=True, stop=True,
                perf_mode=mybir.MatmulPerfMode.DoubleRow,
            )

        def copyback(dst, src, bname, engine_copy):
            """psum->sbuf copy honoring the bias mode for `bname`."""
            kind, val = modes[bname]
            if kind == "zero":
                engine_copy(dst, src)
            else:
                nc.scalar.activation(dst, src, AF.Identity, bias=BV[bname])

        def phase1_last(st):
            """phase1 for the final sample: the dn/av DoubleRow accumulations
            ride along inside the exp loop (pden/pav/ph banks are free once
            the previous sample's phase23 has been emitted), so only
            recip/mul and the gated tail remain after the last exp."""
            xT, xf8_nat = st["xT"], st["xf8_nat"]
            rb, attnT = st["rb"], st["attnT"]

            p_y = pw.tile([128, N], f32, tag="pw")
            nc.tensor.matmul(p_y[:, 0:512], W["Wm"], xT[:, 0:512], start=True, stop=True)
            nc.tensor.matmul(p_y[:, 512:1024], W["Wm"], xT[:, 512:1024], start=True, stop=True)
            yT = sb.tile([128, N], bf16, tag="yT")
            kind, _ = modes["by"]
            if kind == "zero":
                nc.vector.tensor_copy(yT[:, 0:512], p_y[:, 0:512])
                nc.vector.tensor_copy(yT[:, 512:1024], p_y[:, 512:1024])
            else:
                nc.scalar.activation(yT[:, 0:512], p_y[:, 0:512], AF.Identity, bias=BV["by"])
                nc.scalar.activation(yT[:, 512:1024], p_y[:, 512:1024], AF.Identity, bias=BV["by"])

            dn0 = pden.tile([128, 512], f32, tag="pden", name="dn0")
            dn1 = pav.tile([128, 512], f32, tag="pav", name="dn1")
            av0 = ph.tile([128, 512], f32, tag="pwh", name="av0")
            av1 = ph.tile([128, 512], f32, tag="pwh", name="av1")
            expw = expp.tile([128, NT, N], f8, tag="expw")
            for c in range(NT):
                p_l = pw.tile([128, N], f32, tag="pw")
                yTc = yT[:, c * 128:(c + 1) * 128]
                nc.tensor.matmul(p_l[:, 0:512], yTc, xT[:, 0:512], start=True, stop=True)
                nc.tensor.matmul(p_l[:, 512:1024], yTc, xT[:, 512:1024], start=True, stop=True)
                nc.scalar.activation(expw[:, c, :], p_l, AF.Exp, scale=s, bias=expbias)
                if c % 2 == 1:
                    p = c // 2
                    st0, sp0 = p == 0, p == NT // 2 - 1
                    ch = expw[:, c - 1:c + 1, :]
                    nc.tensor.matmul(dn0, ones_dr, ch[:, :, 0:512], start=st0, stop=sp0,
                                     perf_mode=mybir.MatmulPerfMode.DoubleRow)
                    nc.tensor.matmul(dn1, ones_dr, ch[:, :, 512:1024], start=st0, stop=sp0,
                                     perf_mode=mybir.MatmulPerfMode.DoubleRow)
                    xc = xf8_nat[:, c - 1:c + 1, :]
                    nc.tensor.matmul(av0, xc, ch[:, :, 0:512], start=st0, stop=sp0,
                                     perf_mode=mybir.MatmulPerfMode.DoubleRow)
                    nc.tensor.matmul(av1, xc, ch[:, :, 512:1024], start=st0, stop=sp0,
                                     perf_mode=mybir.MatmulPerfMode.DoubleRow)
            nc.vector.reciprocal_approx_fast(rb[:, 0:512], dn0)
            nc.vector.tensor_mul(attnT[:, 0:512], av0, rb[:, 0:512])
            nc.vector.reciprocal_approx_fast(rb[:, 512:1024], dn1)
            nc.vector.tensor_mul(attnT[:, 512:1024], av1, rb[:, 512:1024])
            st["expw"] = expw

        def phase1(st, fillers={}):
            """y projection, QK^T + exp. `fillers` are emit-callbacks (the
            previous sample's attention matmul groups) interleaved between
            logit chunks so the PE stream never drains while ACT catches up."""
            xT = st["xT"]

            p_y = pw.tile([128, N], f32, tag="pw")
            nc.tensor.matmul(p_y[:, 0:512], W["Wm"], xT[:, 0:512], start=True, stop=True)
            nc.tensor.matmul(p_y[:, 512:1024], W["Wm"], xT[:, 512:1024], start=True, stop=True)
            yT = sb.tile([128, N], bf16, tag="yT")
            kind, _ = modes["by"]
            if kind == "zero":
                nc.vector.tensor_copy(yT[:, 0:512], p_y[:, 0:512])
                nc.vector.tensor_copy(yT[:, 512:1024], p_y[:, 512:1024])
            else:
                nc.scalar.activation(yT[:, 0:512], p_y[:, 0:512], AF.Identity, bias=BV["by"])
                nc.scalar.activation(yT[:, 512:1024], p_y[:, 512:1024], AF.Identity, bias=BV["by"])

            expw = expp.tile([128, NT, N], f8, tag="expw")  # [m', c_m, q]
            for c in range(NT):
                p_l = pw.tile([128, N], f32, tag="pw")
                yTc = yT[:, c * 128:(c + 1) * 128]
                nc.tensor.matmul(p_l[:, 0:512], yTc, xT[:, 0:512], start=True, stop=True)
                nc.tensor.matmul(p_l[:, 512:1024], yTc, xT[:, 512:1024], start=True, stop=True)
                nc.scalar.activation(expw[:, c, :], p_l, AF.Exp, scale=s, bias=expbias)
                for f in fillers.get(c, ()):
                    f()
            for c in sorted(fillers):
                if c >= NT:
                    for f in fillers[c]:
                        f()
            st["expw"] = expw

        def all_fillers(b, st):
            """six emit-callbacks covering the whole post-exp pipeline of one
            sample: dn(h0), av(h0), tail(h0), dn(h1), av(h1), tail(h1)."""
            xf8_nat = st["xf8_nat"]
            rb = sb.tile([128, N], f32, tag="rb")
            attnT = sb.tile([128, N], bf16, tag="attnT")
            st["rb"], st["attnT"], st["b"] = rb, attnT, b
            st["u"] = sb.tile([128, N], f32, tag="u", name="u")
            st["gp"] = sb.tile([128, N], bf16, tag="gp", name="gp")
            st["th"] = sb.tile([128, N], bf16, tag="th", name="th")
            st["dlt"] = sb.tile([128, N], bf16, tag="dlt", name="dlt")
            st["dlt_nat"] = sb.tile([128, NT, 128], bf16, tag="dlt_nat", name="dlt_nat")
            st["o"] = sb.tile([128, NT, D], f32, tag="o", name="o")

            def mk_dn(h):
                def emit():
                    expw = st["expw"]
                    sl = slice(h * 512, (h + 1) * 512)
                    p_dn = pden.tile([128, 512], f32, tag="pden")
                    for c in range(NT // 2):
                        nc.tensor.matmul(
                            p_dn, ones_dr, expw[:, 2 * c:2 * c + 2, sl],
                            start=(c == 0), stop=(c == NT // 2 - 1),
                            perf_mode=mybir.MatmulPerfMode.DoubleRow,
                        )
                    nc.vector.reciprocal_approx_fast(rb[:, sl], p_dn)
                return emit

            def mk_av(h):
                def emit():
                    expw = st["expw"]
                    sl = slice(h * 512, (h + 1) * 512)
                    p_av = pav.tile([128, 512], f32, tag="pav")
                    for c in range(NT // 2):
                        nc.tensor.matmul(
                            p_av, xf8_nat[:, 2 * c:2 * c + 2, :], expw[:, 2 * c:2 * c + 2, sl],
                            start=(c == 0), stop=(c == NT // 2 - 1),
                            perf_mode=mybir.MatmulPerfMode.DoubleRow,
                        )
                    nc.vector.tensor_mul(attnT[:, sl], p_av, rb[:, sl])
                return emit

            def mk_ug(h):
                def emit():
                    xT, attnT = st["xT"], st["attnT"]
                    u, gp = st["u"], st["gp"]
                    sl = slice(h * 512, (h + 1) * 512)

                    p_m = ph.tile([128, 512], f32, tag="pwh")
                    nc.tensor.matmul(p_m, W["Woh"], attnT[:, sl], start=True, stop=False)
                    nc.tensor.matmul(p_m, W["Wo1mh"], xT[:, sl], start=False, stop=True)
                    copyback(u[:, sl], p_m, "bo_uh", nc.vector.tensor_copy)

                    p_g = ph.tile([128, 512], f32, tag="pwh")
                    nc.tensor.matmul(p_g, W["Wg1"], xT[:, sl], start=True, stop=False)
                    nc.tensor.matmul(p_g, W["Wog2"], attnT[:, sl], start=False, stop=True)
                    if modes["bo_g"][0] == "zero":
                        nc.vector.tensor_scalar(gp[:, sl], p_g, 0.0, None, op0=OP.max)
                    else:
                        nc.scalar.activation(gp[:, sl], p_g, AF.Relu, bias=BV["bo_g"])
                return emit

            def mk_fin(h):
                def emit():
                    x_nat = st["x_nat"]
                    u, gp, thh, dlt = st["u"], st["gp"], st["th"], st["dlt"]
                    dlt_nat, o = st["dlt_nat"], st["o"]
                    b = st["b"]
                    out_r = out_d[b].rearrange("(c p) d -> p c d", p=128)
                    H = NT // 2
                    tanh_bias = BV["bg3h"] if "bg3h" in BV else 0.0
                    sl = slice(h * 512, (h + 1) * 512)
                    cs = slice(h * H, (h + 1) * H)

                    p_g3 = ph.tile([128, 512], f32, tag="pwh")
                    nc.tensor.matmul(p_g3, W["Wg3"], gp[:, sl], start=True, stop=True)
                    nc.scalar.activation(thh[:, sl], p_g3, AF.Tanh, scale=0.5, bias=tanh_bias)
                    nc.vector.scalar_tensor_tensor(
                        dlt[:, sl], thh[:, sl], 1.0, u[:, sl], op0=OP.add, op1=OP.mult
                    )
                    last = b == BPC - 1
                    teng = nc.scalar if (last and h == 1) else nc.sync
                    teng.dma_start_transpose(dlt_nat[:, cs, :], dlt[:, sl])
                    if last:
                        nc.vector.tensor_add(o[:, cs, :], dlt_nat[:, cs, :], x_nat[:, cs, :])
                        nc.sync.dma_start(out_r[:, cs, :], o[:, cs, :])
                    else:
                        nc.gpsimd.tensor_add(o[:, cs, :], dlt_nat[:, cs, :], x_nat[:, cs, :])
                        nc.gpsimd.dma_start(out_r[:, cs, :], o[:, cs, :])
                return emit

            # slot schedule keyed by logit-chunk index; producers always lead
            # their consumers by >=2 chunks so no engine's in-order queue
            # stalls on a cross-engine round trip.
            return {
                1: [mk_dn(0)], 2: [mk_av(0)], 3: [mk_dn(1)], 4: [mk_ug(0)],
                5: [mk_av(1)], 6: [mk_fin(0)], 7: [mk_ug(1)],
                8: [mk_fin(1)],
            }

        # Software pipeline: emit P23(k-2), P1(k-1), Load(k) per step so each
        # engine's in-order stream interleaves two samples and input DMAs run
        # a full step ahead of first use.
        def run_fillers(fdict, keys=None):
            for c in sorted(fdict):
                if keys is None or c in keys:
                    for f in fdict[c]:
                        f()

        for k in range(1, BPC):
            phase1(ST[k - 1], {})
            if 0 <= k - 2:
                run_fillers(all_fillers(k - 2, ST[k - 2]))
            ST[k] = load(k)
        # final step: emit sample BPC-2's full phase23 first (frees the
        # attention PSUM banks), then the self-accumulating last phase1,
        # then only its tail remains.
        f_prev = all_fillers(BPC - 2, ST[BPC - 2])
        run_fillers(f_prev, keys={1, 2, 3, 4, 5, 7})
        f_last = all_fillers(BPC - 1, ST[BPC - 1])
        phase1_last(ST[BPC - 1])
        run_fillers(f_prev, keys={6, 8})
        run_fillers(f_last, keys={4, 6, 7, 8})

    # Force Exp and Tanh to resolve to the one table set that holds both
    # (exp_and_others): contents-only lie to the set chooser, dict order
    # (= act_func_set_id) preserved; the set actually loaded at runtime does
    # contain both functions (plus Identity/Relu used by bias fallbacks).
    import concourse.bacc as bacc_mod

    real_get = bacc_mod.get_activation_tables
    target = "exp_and_others"

    def patched_get(arch):
        tabs = real_get(arch)
        strip = {AF.Exp, AF.Tanh}
        return {
            name: (set(fns) if name == target else set(fns) - strip)
            for name, fns in tabs.items()
        }

    bacc_mod.get_activation_tables = patched_get
    try:
        nc.compile()
    finally:
        bacc_mod.get_activation_tables = real_get
    return nc


def _prep_host(inputs):
    """Host-side: fold weights/biases; returns (f32 inputs, wpack bf16, biases)."""
    f32 = np.float32
    g = {k: np.asarray(v, f32) for k, v in inputs.items()}

    Wm = g["Wk"] @ g["Wq"].T                       # y = x@Wk@Wq^T; logit=x_i.y_j
    Wvo = g["Wv"] @ g["Wo"]                        # v path folded into tail
    Wo1m = g["Wo1"] - np.eye(D, dtype=f32)
    Wog2 = Wvo @ g["Wg2"]                          # msg path folded into gate
    bo_msg = g["bo"] + g["bv"] @ g["Wo"]           # bv folded through Wo
    bo_uh = 0.5 * (bo_msg + g["bo1"])              # msg bias + ret bias, halved
    bo_g = bo_msg @ g["Wg2"] + g["bg1"] + g["bg2"]
    bg3h = 0.5 * g["bg3"]                          # tanh((z+bg3)/2) bias
    by = g["Wk"] @ g["bq"]                         # per-key logit bias

    wmap = {
        "Wm": Wm, "Woh": 0.5 * Wvo, "Wo1mh": 0.5 * Wo1m,
        "Wg1": g["Wg1"], "Wog2": Wog2, "Wg3": g["Wg3"],
    }
    bmap = {
        "by": by,
        "bo_uh": bo_uh, "bo_g": bo_g, "bg3h": bg3h,
    }
    bf16 = ml_dtypes.bfloat16
    wpack = np.stack([wmap[n] for n in WNAMES], axis=1).astype(bf16)
    return g, np.ascontiguousarray(wpack), bmap


def _prep_inputs(inputs):
    g, wpack, bmap = _prep_host(inputs)
    modes = {n: _bias_mode(v) for n, v in bmap.items()}
    base = {"wpack": wpack}
    for n, v in bmap.items():
        if modes[n][0] == "ap":
            base[n] = np.ascontiguousarray(v.reshape(D, 1).astype(np.float32))
    xbf = np.ascontiguousarray(g["x"].astype(ml_dtypes.bfloat16))
    xtb = np.ascontiguousarray(np.swapaxes(xbf, 1, 2))
    xf8 = np.ascontiguousarray(g["x"].astype(ml_dtypes.float8_e4m3fn))
    in_maps = []
    for c in range(NCORES):
        m = dict(base)
        m["xbf"] = np.ascontiguousarray(xbf[c * BPC:(c + 1) * BPC])
        m["xtb"] = np.ascontiguousarray(xtb[c * BPC:(c + 1) * BPC])
        m["xf8"] = np.ascontiguousarray(xf8[c * BPC:(c + 1) * BPC])
        in_maps.append(m)
    return in_maps, modes


def kernel(**inputs):
    from concourse.bass_utils import run_bass_kernel_spmd

    in_maps, modes = _prep_inputs(inputs)
    key = tuple(sorted((n, k[0], k[1]) for n, k in modes.items()))
    if _CACHE.get("key") != key:
        _CACHE["nc"] = _build_nc(modes)
        _CACHE["key"] = key
    nc = _CACHE["nc"]

    res = run_bass_kernel_spmd(nc, in_maps, list(range(NCORES)))
    out = np.concatenate([r["out"] for r in res.results], axis=0)
    return out.astype(np.float32)


# revision 18
# speedup vs baseline: 1.0583x; 1.0583x over previous
"""Trainium2 Bass kernel for nn_Net_5488968204310 (gnn_message_passing).

Single-head self-attention (D=128) over N=1024 nodes + gated residual update,
batch B=32, data-parallel across 8 NeuronCores (4 samples per core).

Design notes:
  - "T layout" (features d on partitions, nodes on free dim) for every matmul;
    contraction is always over d.
  - QK fold: logit_ij = x_i.(M^T x_j) with M = Wq@Wk^T folded on the host, so
    only ONE projection matmul (stationary S = Wk@Wq^T) is needed and the
    logits' moving operand is xT itself. The per-query bias term is
    softmax-invariant and dropped; the per-key term folds into y's bias.
  - V fold: attn@Wo = (A@X)@(Wv@Wo). Attention runs over raw X (host-cast
    fp8 upload, natural layout) and Wv folds into the tail weights, removing
    the v projection and its PSUM->SBUF cast.
  - QK^T: logitsT chunks [m_chunk(128) x q(1024)] = yT_chunk.T @ xT; exp() on
    the scalar engine straight from PSUM with the 1/sqrt(D) scale and a -2
    bias folded in (rescales exp into fp8 range; the uniform e^-2 factor
    cancels between numerator and denominator).
  - A@X keeps xf8 as the stationary operand (DoubleRow fp8, dense 512-col
    streams). The softmax denominator is a parallel ones.T @ expw DoubleRow
    accumulation; 1/denom via the DVE reciprocal_approx_fast custom op.
  - gate sigmoid as 0.5*(1+tanh(z/2)): one ACT op (Tanh) instead of a 3-op
    exp/ln chain. The 0.5 gate factor is folded into the u-path weights on
    the host so dlt = u_half * (tanh+1) is a single DVE scalar_tensor_tensor
    reading u DIRECTLY from PSUM (parked in the idle dn/av banks during the
    tail) -- no u copyback instruction at all.
  - ACT functions used (Exp, Tanh, Identity/Relu fallbacks) all live in the
    exp_and_others table set, enforced by a scoped patch of the table
    metadata at compile time: exactly one ACT_TABLE_LOAD per run.
  - x ships three ways from the host (bf16 natural, bf16 pre-transposed,
    fp8 natural) so no on-chip transposes or casts sit on the input path;
    the six folded 128x128 weights ship as ONE packed DMA.
  - A burst of dummy DoubleRow matmuls on the ones constant warms the PE out
    of its cold p-state while the first x DMAs are still in flight.
  - The residual add runs in bf16 on the Pool engine, which also issues the
    output stores so they chain straight after the add with no engine hop.
"""

import math

import numpy as np
import ml_dtypes

B, N, D = 32, 1024, 128
NCORES = 8
BPC = B // NCORES  # samples per core
NT = N // 128      # node chunks per sample

WNAMES = ["Wm", "Woh", "Wo1mh", "Wg1", "Wog2", "Wg3"]

_CACHE = {}


def _bias_mode(vec):
    """(kind, value) where kind in {'zero', 'uniform', 'ap'}."""
    v = np.asarray(vec, np.float32)
    if not np.any(v):
        return ("zero", 0.0)
    if np.all(v == v.flat[0]):
        return ("uniform", float(v.flat[0]))
    return ("ap", 0.0)


def _build_nc(modes):
    import concourse.bacc as bacc
    import concourse.tile as tile
    from concourse import mybir
    from contextlib import ExitStack

    f32 = mybir.dt.float32
    bf16 = mybir.dt.bfloat16
    f8 = mybir.dt.float8e4
    AF = mybir.ActivationFunctionType
    OP = mybir.AluOpType
    DR = mybir.MatmulPerfMode.DoubleRow

    nc = bacc.Bacc("TRN2", target_bir_lowering=False, debug=False)

    xb_d = nc.dram_tensor("xbf", [BPC, N, D], bf16, kind="ExternalInput")
    xt_d = nc.dram_tensor("xtb", [BPC, D, N], bf16, kind="ExternalInput")
    xf8_d = nc.dram_tensor("xf8", [BPC, N, D], f8, kind="ExternalInput")
    out_d = nc.dram_tensor("out", [BPC, N, D], f32, kind="ExternalOutput")
    wp_d = nc.dram_tensor("wpack", [D, len(WNAMES), D], bf16, kind="ExternalInput")
    b_d = {
        n: nc.dram_tensor(n, [D, 1], f32, kind="ExternalInput")
        for n in modes if modes[n][0] == "ap"
    }

    s = 1.0 / math.sqrt(D)

    with tile.TileContext(nc) as tc, ExitStack() as ctx:
        consts = ctx.enter_context(tc.tile_pool(name="consts", bufs=1))
        sb = ctx.enter_context(tc.tile_pool(name="sb", bufs=2))
        sb3 = ctx.enter_context(tc.tile_pool(name="sb3", bufs=3))
        expp = ctx.enter_context(tc.tile_pool(name="expp", bufs=2))
        pw = ctx.enter_context(tc.tile_pool(name="pw", bufs=2, space="PSUM"))
        ph = ctx.enter_context(tc.tile_pool(name="ph", bufs=2, space="PSUM"))
        pav = ctx.enter_context(tc.tile_pool(name="pav", bufs=1, space="PSUM"))
        pden = ctx.enter_context(tc.tile_pool(name="pden", bufs=1, space="PSUM"))

        ST = {}

        def load(b):
            """input DMAs for sample b (issued one pipeline step early)."""
            st = {}
            xT = sb3.tile([128, N], bf16, tag="xT")  # [d, n]
            nc.sync.dma_start(xT, xt_d[b])
            x_nat = sb3.tile([128, NT, D], bf16, tag="x_nat")
            nc.sync.dma_start(x_nat, xb_d[b].rearrange("(c p) d -> p c d", p=128))
            xf8_nat = sb3.tile([128, NT, D], f8, tag="xf8_nat")
            nc.sync.dma_start(xf8_nat, xf8_d[b].rearrange("(c p) d -> p c d", p=128))
            st["x_nat"], st["xf8_nat"], st["xT"] = x_nat, xf8_nat, xT
            return st

        # input DMAs for sample 0 go out before anything else; the packed
        # weight DMA + consts follow on other engines so nothing serializes
        # behind the sequencer's per-DMA issue cost.
        ST[0] = load(0)

        wpack = consts.tile([D, len(WNAMES), D], bf16, tag="wpack")
        nc.scalar.dma_start(wpack, wp_d[:, :, :])
        W = {n: wpack[:, i, :] for i, n in enumerate(WNAMES)}
        ones_dr = consts.tile([128, 2, 128], f8, tag="ones_dr")
        nc.gpsimd.memset(ones_dr, 1.0)
        expbias = consts.tile([128, 1], f32, tag="expbias")
        nc.gpsimd.memset(expbias, -2.0)
        BV = {}
        for n in b_d:
            t = consts.tile([D, 1], f32, tag=f"b_{n}")
            nc.scalar.dma_start(t, b_d[n][:, :])
            BV[n] = t
        for n, (kind, val) in modes.items():
            if kind == "uniform":
                t = consts.tile([D, 1], f32, tag=f"b_{n}")
                nc.gpsimd.memset(t, val)
                BV[n] = t

        # PE p-state warmup: dummy DoubleRow matmuls on the ones const while
        # the first x DMAs are still in flight, so the first real matmuls run
        # at full clock instead of the cold 0.65 GHz p-state.
        pdum = pden.tile([128, 512], f32, tag="pden", name="pdum")
        for _ in range(16):
            nc.tensor.matmul(pdum[:, 0:128], ones_dr, ones_dr,
                             start=True, stop=True, perf_mode=DR)

        def phase1(st):
            """y projection, QK^T + exp."""
            xT = st["xT"]

            p_y = pw.tile([128, N], f32, tag="pw")
            nc.tensor.matmul(p_y[:, 0:512], W["Wm"], xT[:, 0:512], start=True, stop=True)
            nc.tensor.matmul(p_y[:, 512:1024], W["Wm"], xT[:, 512:1024], start=True, stop=True)
            yT = sb.tile([128, N], bf16, tag="yT")
            if modes["by"][0] == "zero":
                nc.vector.tensor_copy(yT[:, 0:512], p_y[:, 0:512])
                nc.vector.tensor_copy(yT[:, 512:1024], p_y[:, 512:1024])
            else:
                nc.scalar.activation(yT[:, 0:512], p_y[:, 0:512], AF.Identity, bias=BV["by"])
                nc.scalar.activation(yT[:, 512:1024], p_y[:, 512:1024], AF.Identity, bias=BV["by"])

            expw = expp.tile([128, NT, N], f8, tag="expw")  # [m', c_m, q]
            for c in range(NT):
                p_l = pw.tile([128, N], f32, tag="pw")
                yTc = yT[:, c * 128:(c + 1) * 128]
                nc.tensor.matmul(p_l[:, 0:512], yTc, xT[:, 0:512], start=True, stop=True)
                nc.tensor.matmul(p_l[:, 512:1024], yTc, xT[:, 512:1024], start=True, stop=True)
                nc.scalar.activation(expw[:, c, :], p_l, AF.Exp, scale=s, bias=expbias)
            st["expw"] = expw

        def phase23(b, st):
            """softmax normalize + gated update tail; store.

            u = 0.5*(ret - x) stays in PSUM (parked in the dn/av banks) and
            is consumed directly by the dlt scalar_tensor_tensor;
            gate = 0.5*(1 + tanh((z+bg3)/2)); out = x + u*(1+tanh)."""
            expw, xf8_nat = st["expw"], st["xf8_nat"]
            x_nat, xT = st["x_nat"], st["xT"]
            rb = sb.tile([128, N], f32, tag="rb")
            attnT = sb.tile([128, N], bf16, tag="attnT")
            gp = sb.tile([128, N], bf16, tag="gp")
            th = sb.tile([128, N], bf16, tag="th")
            dlt = sb.tile([128, N], bf16, tag="dlt")
            dlt_nat = sb.tile([128, NT, 128], bf16, tag="dlt_nat")
            o = sb.tile([128, NT, D], f32, tag="o")
            out_r = out_d[b].rearrange("(c p) d -> p c d", p=128)
            H = NT // 2
            last = b == BPC - 1
            tanh_bias = BV["bg3h"] if "bg3h" in BV else 0.0

            for h in range(2):
                sl = slice(h * 512, (h + 1) * 512)
                p_dn = pden.tile([128, 512], f32, tag="pden")
                for c in range(NT // 2):
                    nc.tensor.matmul(p_dn, ones_dr, expw[:, 2 * c:2 * c + 2, sl],
                                     start=(c == 0), stop=(c == NT // 2 - 1), perf_mode=DR)
                nc.vector.reciprocal_approx_fast(rb[:, sl], p_dn)
                p_av = pav.tile([128, 512], f32, tag="pav")
                for c in range(NT // 2):
                    nc.tensor.matmul(p_av, xf8_nat[:, 2 * c:2 * c + 2, :],
                                     expw[:, 2 * c:2 * c + 2, sl],
                                     start=(c == 0), stop=(c == NT // 2 - 1), perf_mode=DR)
                nc.vector.tensor_mul(attnT[:, sl], p_av, rb[:, sl])

            upool = [pden, pav]
            utags = ["pden", "pav"]
            for h in range(2):
                sl = slice(h * 512, (h + 1) * 512)
                cs = slice(h * H, (h + 1) * H)

                p_m = upool[h].tile([128, 512], f32, tag=utags[h], name="p_m")
                nc.tensor.matmul(p_m, W["Woh"], attnT[:, sl], start=True, stop=False)
                nc.tensor.matmul(p_m, W["Wo1mh"], xT[:, sl], start=False, stop=True)
                if modes["bo_uh"][0] != "zero":
                    nc.scalar.activation(p_m, p_m, AF.Identity, bias=BV["bo_uh"])

                p_g = ph.tile([128, 512], f32, tag="pwh")
                nc.tensor.matmul(p_g, W["Wg1"], xT[:, sl], start=True, stop=False)
                nc.tensor.matmul(p_g, W["Wog2"], attnT[:, sl], start=False, stop=True)
                if modes["bo_g"][0] == "zero":
                    nc.vector.tensor_scalar(gp[:, sl], p_g, 0.0, None, op0=OP.max)
                else:
                    nc.scalar.activation(gp[:, sl], p_g, AF.Relu, bias=BV["bo_g"])

                p_g3 = ph.tile([128, 512], f32, tag="pwh")
                nc.tensor.matmul(p_g3, W["Wg3"], gp[:, sl], start=True, stop=True)
                nc.scalar.activation(th[:, sl], p_g3, AF.Tanh, scale=0.5, bias=tanh_bias)
                nc.vector.scalar_tensor_tensor(
                    dlt[:, sl], th[:, sl], 1.0, p_m, op0=OP.add, op1=OP.mult
                )
                teng = nc.scalar if (last and h == 1) else nc.sync
                teng.dma_start_transpose(dlt_nat[:, cs, :], dlt[:, sl])
                nc.gpsimd.tensor_add(o[:, cs, :], dlt_nat[:, cs, :], x_nat[:, cs, :])
                nc.gpsimd.dma_start(out_r[:, cs, :], o[:, cs, :])

        # Software pipeline: emit P1(k-1), P23(k-2), Load(k) per step so each
        # engine's in-order stream interleaves two samples and input DMAs run
        # a full step ahead of first use.
        for k in range(1, BPC + 2):
            if 0 <= k - 1 < BPC:
                phase1(ST[k - 1])
            if 0 <= k - 2:
                phase23(k - 2, ST[k - 2])
            if k < BPC:
                ST[k] = load(k)

    # Force Exp and Tanh to resolve to the one table set that holds both
    # (exp_and_others): contents-only lie to the set chooser, dict order
    # (= act_func_set_id) preserved; the set actually loaded at runtime does
    # contain both functions (plus Identity/Relu used by bias fallbacks).
    import concourse.bacc as bacc_mod

    real_get = bacc_mod.get_activation_tables
    target = "exp_and_others"

    def patched_get(arch):
        tabs = real_get(arch)
        strip = {AF.Exp, AF.Tanh}
        return {
            name: (set(fns) if name == target else set(fns) - strip)
            for name, fns in tabs.items()
        }

    bacc_mod.get_activation_tables = patched_get
    try:
        nc.compile()
    finally:
        bacc_mod.get_activation_tables = real_get
    return nc


def _prep_host(inputs):
    """Host-side: fold weights/biases; returns (f32 inputs, wpack bf16, biases)."""
    f32 = np.float32
    g = {k: np.asarray(v, f32) for k, v in inputs.items()}

    Wm = g["Wk"] @ g["Wq"].T                       # y = x@Wk@Wq^T; logit=x_i.y_j
    Wvo = g["Wv"] @ g["Wo"]                        # v path folded into tail
    Wo1m = g["Wo1"] - np.eye(D, dtype=f32)
    Wog2 = Wvo @ g["Wg2"]                          # msg path folded into gate
    bo_msg = g["bo"] + g["bv"] @ g["Wo"]           # bv folded through Wo
    bo_uh = 0.5 * (bo_msg + g["bo1"])              # msg bias + ret bias, halved
    bo_g = bo_msg @ g["Wg2"] + g["bg1"] + g["bg2"]
    bg3h = 0.5 * g["bg3"]                          # tanh((z+bg3)/2) bias
    by = g["Wk"] @ g["bq"]                         # per-key logit bias

    wmap = {
        "Wm": Wm, "Woh": 0.5 * Wvo, "Wo1mh": 0.5 * Wo1m,
        "Wg1": g["Wg1"], "Wog2": Wog2, "Wg3": g["Wg3"],
    }
    bmap = {
        "by": by,
        "bo_uh": bo_uh, "bo_g": bo_g, "bg3h": bg3h,
    }
    bf16 = ml_dtypes.bfloat16
    wpack = np.stack([wmap[n] for n in WNAMES], axis=1).astype(bf16)
    return g, np.ascontiguousarray(wpack), bmap


def _prep_inputs(inputs):
    g, wpack, bmap = _prep_host(inputs)
    modes = {n: _bias_mode(v) for n, v in bmap.items()}
    base = {"wpack": wpack}
    for n, v in bmap.items():
        if modes[n][0] == "ap":
            base[n] = np.ascontiguousarray(v.reshape(D, 1).astype(np.float32))
    xbf = np.ascontiguousarray(g["x"].astype(ml_dtypes.bfloat16))
    xtb = np.ascontiguousarray(np.swapaxes(xbf, 1, 2))
    xf8 = np.ascontiguousarray(g["x"].astype(ml_dtypes.float8_e4m3fn))
    in_maps = []
    for c in range(NCORES):
        m = dict(base)
        m["xbf"] = np.ascontiguousarray(xbf[c * BPC:(c + 1) * BPC])
        m["xtb"] = np.ascontiguousarray(xtb[c * BPC:(c + 1) * BPC])
        m["xf8"] = np.ascontiguousarray(xf8[c * BPC:(c + 1) * BPC])
        in_maps.append(m)
    return in_maps, modes


def kernel(**inputs):
    from concourse.bass_utils import run_bass_kernel_spmd

    in_maps, modes = _prep_inputs(inputs)
    key = tuple(sorted((n, k[0], k[1]) for n, k in modes.items()))
    if _CACHE.get("key") != key:
        _CACHE["nc"] = _build_nc(modes)
        _CACHE["key"] = key
    nc = _CACHE["nc"]

    res = run_bass_kernel_spmd(nc, in_maps, list(range(NCORES)))
    out = np.concatenate([r["out"] for r in res.results], axis=0)
    return out.astype(np.float32)


# revision 19
# speedup vs baseline: 1.0909x; 1.0308x over previous
"""Trainium2 Bass kernel for nn_Net_5488968204310 (gnn_message_passing).

Single-head self-attention (D=128) over N=1024 nodes + gated residual update,
batch B=32, data-parallel across 8 NeuronCores (4 samples per core).

Design notes:
  - "T layout" (features d on partitions, nodes on free dim) for every matmul;
    contraction is always over d.
  - QK fold: logit_ij = x_i.(M^T x_j) with M = Wq@Wk^T folded on the host, so
    only ONE projection matmul (stationary S = Wk@Wq^T) is needed and the
    logits' moving operand is xT itself. The per-query bias term is
    softmax-invariant and dropped; the per-key term folds into y's bias.
  - V fold: attn@Wo = (A@X)@(Wv@Wo). Attention runs over raw X (host-cast
    fp8 upload, natural layout) and Wv folds into the tail weights, removing
    the v projection and its PSUM->SBUF cast.
  - QK^T: logitsT chunks [m_chunk(128) x q(1024)] = yT_chunk.T @ xT; exp() on
    the scalar engine straight from PSUM with the 1/sqrt(D) scale and a -2
    bias folded in (rescales exp into fp8 range; the uniform e^-2 factor
    cancels between numerator and denominator).
  - A@X keeps xf8 as the stationary operand (DoubleRow fp8, dense 512-col
    streams). The softmax denominator is a parallel ones.T @ expw DoubleRow
    accumulation; 1/denom via the DVE reciprocal_approx_fast custom op.
  - gate sigmoid as 0.5*(1+tanh(z/2)): one ACT op (Tanh) instead of a 3-op
    exp/ln chain. The 0.5 gate factor is folded into the u-path weights on
    the host so dlt = u_half * (tanh+1) is a single DVE scalar_tensor_tensor
    reading u DIRECTLY from PSUM (parked in the idle dn/av banks during the
    tail) -- no u copyback instruction at all.
  - ACT functions used (Exp, Tanh, Identity/Relu fallbacks) all live in the
    exp_and_others table set, enforced by a scoped patch of the table
    metadata at compile time: exactly one ACT_TABLE_LOAD per run.
  - x ships three ways from the host (bf16 natural, bf16 pre-transposed,
    fp8 natural) so no on-chip transposes or casts sit on the input path;
    the six folded 128x128 weights ship as ONE packed DMA.
  - A burst of dummy DoubleRow matmuls on the ones constant warms the PE out
    of its cold p-state while the first x DMAs are still in flight.
  - The residual add runs in bf16 on the Pool engine, which also issues the
    output stores so they chain straight after the add with no engine hop.
"""

import math

import numpy as np
import ml_dtypes

B, N, D = 32, 1024, 128
NCORES = 8
BPC = B // NCORES  # samples per core
NT = N // 128      # node chunks per sample

WNAMES = ["Wm", "Woh", "Wo1mh", "Wg1", "Wog2", "Wg3"]

_CACHE = {}


def _bias_mode(vec):
    """(kind, value) where kind in {'zero', 'uniform', 'ap'}."""
    v = np.asarray(vec, np.float32)
    if not np.any(v):
        return ("zero", 0.0)
    if np.all(v == v.flat[0]):
        return ("uniform", float(v.flat[0]))
    return ("ap", 0.0)


def _build_nc(modes):
    import concourse.bacc as bacc
    import concourse.tile as tile
    from concourse import mybir
    from contextlib import ExitStack

    f32 = mybir.dt.float32
    bf16 = mybir.dt.bfloat16
    f8 = mybir.dt.float8e4
    AF = mybir.ActivationFunctionType
    OP = mybir.AluOpType
    DR = mybir.MatmulPerfMode.DoubleRow

    nc = bacc.Bacc("TRN2", target_bir_lowering=False, debug=False)

    xb_d = nc.dram_tensor("xbf", [BPC, N, D], bf16, kind="ExternalInput")
    xt_d = nc.dram_tensor("x16t", [BPC, D, N], f8, kind="ExternalInput")
    xf8_d = nc.dram_tensor("xf8", [BPC, N, D], f8, kind="ExternalInput")
    out_d = nc.dram_tensor("out", [BPC, N, D], f32, kind="ExternalOutput")
    wp_d = nc.dram_tensor("wpack8", [D, 5, D], f8, kind="ExternalInput")
    wg3_d = nc.dram_tensor("wg3", [D, D], bf16, kind="ExternalInput")
    b_d = {
        n: nc.dram_tensor(n, [D, 1], f32, kind="ExternalInput")
        for n in modes if modes[n][0] == "ap"
    }

    s = 1.0 / math.sqrt(D)

    with tile.TileContext(nc) as tc, ExitStack() as ctx:
        consts = ctx.enter_context(tc.tile_pool(name="consts", bufs=1))
        sb = ctx.enter_context(tc.tile_pool(name="sb", bufs=2))
        sb3 = ctx.enter_context(tc.tile_pool(name="sb3", bufs=3))
        expp = ctx.enter_context(tc.tile_pool(name="expp", bufs=2))
        pw = ctx.enter_context(tc.tile_pool(name="pw", bufs=2, space="PSUM"))
        ph = ctx.enter_context(tc.tile_pool(name="ph", bufs=2, space="PSUM"))
        pav = ctx.enter_context(tc.tile_pool(name="pav", bufs=1, space="PSUM"))
        pden = ctx.enter_context(tc.tile_pool(name="pden", bufs=1, space="PSUM"))

        ST = {}

        def load(b):
            """input DMAs for sample b (issued one pipeline step early)."""
            st = {}
            # two-slot tile: slot 0 <- attnX/16 (written by the DVE mul in
            # phase23), slot 1 <- x/16 transposed (DMA). The fp8 DoubleRow
            # tail matmuls consume both slots in one instruction.
            axT = sb3.tile([128, 2, N], f8, tag="axT")  # [d, slot, n]
            nc.sync.dma_start(axT[:, 1, :], xt_d[b])
            x_nat = sb3.tile([128, NT, D], bf16, tag="x_nat")
            nc.sync.dma_start(x_nat, xb_d[b].rearrange("(c p) d -> p c d", p=128))
            xf8_nat = sb3.tile([128, NT, D], f8, tag="xf8_nat")
            nc.sync.dma_start(xf8_nat, xf8_d[b].rearrange("(c p) d -> p c d", p=128))
            st["x_nat"], st["xf8_nat"], st["axT"] = x_nat, xf8_nat, axT
            return st

        # input DMAs for sample 0 go out before anything else; the packed
        # weight DMA + consts follow on other engines so nothing serializes
        # behind the sequencer's per-DMA issue cost.
        ST[0] = load(0)

        wpack = consts.tile([D, 5, D], f8, tag="wpack")
        nc.scalar.dma_start(wpack, wp_d[:, :, :])
        wg3 = consts.tile([D, D], bf16, tag="wg3")
        nc.scalar.dma_start(wg3, wg3_d[:, :])
        Wm, Wu, Wg = wpack[:, 0, :], wpack[:, 1:3, :], wpack[:, 3:5, :]
        # ones = 16 so the softmax denominator absorbs the 1/16 activation
        # scale: attnT = p_av/(16*sum) pairs exactly with the 16x weights.
        ones_dr = consts.tile([128, 2, 128], f8, tag="ones_dr")
        nc.gpsimd.memset(ones_dr, 16.0)
        expbias = consts.tile([128, 1], f32, tag="expbias")
        nc.gpsimd.memset(expbias, -2.0)
        BV = {}
        for n in b_d:
            t = consts.tile([D, 1], f32, tag=f"b_{n}")
            nc.scalar.dma_start(t, b_d[n][:, :])
            BV[n] = t
        for n, (kind, val) in modes.items():
            if kind == "uniform":
                t = consts.tile([D, 1], f32, tag=f"b_{n}")
                nc.gpsimd.memset(t, val)
                BV[n] = t

        # PE p-state warmup: dummy DoubleRow matmuls on the ones const while
        # the first x DMAs are still in flight, so the first real matmuls run
        # at full clock instead of the cold 0.65 GHz p-state.
        pdum = pden.tile([128, 512], f32, tag="pden", name="pdum")
        for _ in range(16):
            nc.tensor.matmul(pdum[:, 0:128], ones_dr, ones_dr,
                             start=True, stop=True, perf_mode=DR)

        def phase1(st):
            """y projection, QK^T + exp (all fp8, 1/16-scaled operands; the
            1/256 logit scale folds into the exp activation scale)."""
            xs = st["axT"][:, 1, :]

            p_y = pw.tile([128, N], f32, tag="pw")
            nc.tensor.matmul(p_y[:, 0:512], Wm, xs[:, 0:512], start=True, stop=True)
            nc.tensor.matmul(p_y[:, 512:1024], Wm, xs[:, 512:1024], start=True, stop=True)
            yT = sb.tile([128, N], f8, tag="yT")
            if modes["by"][0] == "zero":
                nc.vector.tensor_copy(yT[:, 0:512], p_y[:, 0:512])
                nc.vector.tensor_copy(yT[:, 512:1024], p_y[:, 512:1024])
            else:
                nc.scalar.activation(yT[:, 0:512], p_y[:, 0:512], AF.Identity, bias=BV["by"])
                nc.scalar.activation(yT[:, 512:1024], p_y[:, 512:1024], AF.Identity, bias=BV["by"])

            expw = expp.tile([128, NT, N], f8, tag="expw")  # [m', c_m, q]
            for c in range(NT):
                p_l = pw.tile([128, N], f32, tag="pw")
                yTc = yT[:, c * 128:(c + 1) * 128]
                nc.tensor.matmul(p_l[:, 0:512], yTc, xs[:, 0:512], start=True, stop=True)
                nc.tensor.matmul(p_l[:, 512:1024], yTc, xs[:, 512:1024], start=True, stop=True)
                nc.scalar.activation(expw[:, c, :], p_l, AF.Exp, scale=s * 256.0, bias=expbias)
            st["expw"] = expw

        def phase23(b, st):
            """softmax normalize + gated update tail; store.

            u = 0.5*(ret - x) stays in PSUM (parked in the dn/av banks) and
            is consumed directly by the dlt scalar_tensor_tensor;
            gate = 0.5*(1 + tanh((z+bg3)/2)); out = x + u*(1+tanh)."""
            expw, xf8_nat = st["expw"], st["xf8_nat"]
            x_nat, axT = st["x_nat"], st["axT"]
            rb = sb.tile([128, N], f32, tag="rb")
            gp = sb.tile([128, N], bf16, tag="gp")
            th = sb.tile([128, N], bf16, tag="th")
            dlt = sb.tile([128, N], bf16, tag="dlt")
            dlt_nat = sb.tile([128, NT, 128], bf16, tag="dlt_nat")
            o = sb.tile([128, NT, D], f32, tag="o")
            out_r = out_d[b].rearrange("(c p) d -> p c d", p=128)
            H = NT // 2
            last = b == BPC - 1
            tanh_bias = BV["bg3h"] if "bg3h" in BV else 0.0

            for h in range(2):
                sl = slice(h * 512, (h + 1) * 512)
                p_dn = pden.tile([128, 512], f32, tag="pden")
                for c in range(NT // 2):
                    nc.tensor.matmul(p_dn, ones_dr, expw[:, 2 * c:2 * c + 2, sl],
                                     start=(c == 0), stop=(c == NT // 2 - 1), perf_mode=DR)
                nc.vector.reciprocal_approx_fast(rb[:, sl], p_dn)
                p_av = pav.tile([128, 512], f32, tag="pav")
                for c in range(NT // 2):
                    nc.tensor.matmul(p_av, xf8_nat[:, 2 * c:2 * c + 2, :],
                                     expw[:, 2 * c:2 * c + 2, sl],
                                     start=(c == 0), stop=(c == NT // 2 - 1), perf_mode=DR)
                nc.vector.tensor_mul(axT[:, 0, sl], p_av, rb[:, sl])

            upool = [pden, pav]
            utags = ["pden", "pav"]
            for h in range(2):
                sl = slice(h * 512, (h + 1) * 512)
                cs = slice(h * H, (h + 1) * H)

                p_m = upool[h].tile([128, 512], f32, tag=utags[h], name="p_m")
                nc.tensor.matmul(p_m, Wu, axT[:, :, sl], start=True, stop=True, perf_mode=DR)
                if modes["bo_uh"][0] != "zero":
                    nc.scalar.activation(p_m, p_m, AF.Identity, bias=BV["bo_uh"])

                p_g = ph.tile([128, 512], f32, tag="pwh")
                nc.tensor.matmul(p_g, Wg, axT[:, :, sl], start=True, stop=True, perf_mode=DR)
                if modes["bo_g"][0] == "zero":
                    nc.vector.tensor_scalar(gp[:, sl], p_g, 0.0, None, op0=OP.max)
                else:
                    nc.scalar.activation(gp[:, sl], p_g, AF.Relu, bias=BV["bo_g"])

                p_g3 = ph.tile([128, 512], f32, tag="pwh")
                nc.tensor.matmul(p_g3, wg3, gp[:, sl], start=True, stop=True)
                nc.scalar.activation(th[:, sl], p_g3, AF.Tanh, scale=0.5, bias=tanh_bias)
                nc.vector.scalar_tensor_tensor(
                    dlt[:, sl], th[:, sl], 1.0, p_m, op0=OP.add, op1=OP.mult
                )
                teng = nc.scalar if (last and h == 1) else nc.sync
                teng.dma_start_transpose(dlt_nat[:, cs, :], dlt[:, sl])
                nc.gpsimd.tensor_add(o[:, cs, :], dlt_nat[:, cs, :], x_nat[:, cs, :])
                nc.gpsimd.dma_start(out_r[:, cs, :], o[:, cs, :])

        # Software pipeline: emit P1(k-1), P23(k-2), Load(k) per step so each
        # engine's in-order stream interleaves two samples and input DMAs run
        # a full step ahead of first use.
        for k in range(1, BPC + 2):
            if 0 <= k - 1 < BPC:
                phase1(ST[k - 1])
            if 0 <= k - 2:
                phase23(k - 2, ST[k - 2])
            if k < BPC:
                ST[k] = load(k)

    # Force Exp and Tanh to resolve to the one table set that holds both
    # (exp_and_others): contents-only lie to the set chooser, dict order
    # (= act_func_set_id) preserved; the set actually loaded at runtime does
    # contain both functions (plus Identity/Relu used by bias fallbacks).
    import concourse.bacc as bacc_mod

    real_get = bacc_mod.get_activation_tables
    target = "exp_and_others"

    def patched_get(arch):
        tabs = real_get(arch)
        strip = {AF.Exp, AF.Tanh}
        return {
            name: (set(fns) if name == target else set(fns) - strip)
            for name, fns in tabs.items()
        }

    bacc_mod.get_activation_tables = patched_get
    try:
        nc.compile()
    finally:
        bacc_mod.get_activation_tables = real_get
    return nc


def _prep_host(inputs):
    """Host-side: fold weights/biases; returns (f32 inputs, wpack bf16, biases)."""
    f32 = np.float32
    g = {k: np.asarray(v, f32) for k, v in inputs.items()}

    Wm = g["Wk"] @ g["Wq"].T                       # y = x@Wk@Wq^T; logit=x_i.y_j
    Wvo = g["Wv"] @ g["Wo"]                        # v path folded into tail
    Wo1m = g["Wo1"] - np.eye(D, dtype=f32)
    Wog2 = Wvo @ g["Wg2"]                          # msg path folded into gate
    bo_msg = g["bo"] + g["bv"] @ g["Wo"]           # bv folded through Wo
    bo_uh = 0.5 * (bo_msg + g["bo1"])              # msg bias + ret bias, halved
    bo_g = bo_msg @ g["Wg2"] + g["bg1"] + g["bg2"]
    bg3h = 0.5 * g["bg3"]                          # tanh((z+bg3)/2) bias
    by = g["Wk"] @ g["bq"]                         # per-key logit bias

    # fp8 pack: [Wm, 16*Woh, 16*Wo1mh, 16*Wog2, 16*Wg1]; the 16x weight
    # scale cancels against the 1/16-scaled x/attn activations exactly.
    f8 = ml_dtypes.float8_e4m3fn
    wpack8 = np.stack(
        [Wm, 8.0 * Wvo, 8.0 * Wo1m, 16.0 * Wog2, 16.0 * g["Wg1"]], axis=1
    ).astype(f8)
    wg3 = g["Wg3"].astype(ml_dtypes.bfloat16)
    bmap = {
        "by": by / 16.0,
        "bo_uh": bo_uh, "bo_g": bo_g, "bg3h": bg3h,
    }
    return g, (np.ascontiguousarray(wpack8), np.ascontiguousarray(wg3)), bmap


def _prep_inputs(inputs):
    g, (wpack8, wg3), bmap = _prep_host(inputs)
    modes = {n: _bias_mode(v) for n, v in bmap.items()}
    base = {"wpack8": wpack8, "wg3": wg3}
    for n, v in bmap.items():
        if modes[n][0] == "ap":
            base[n] = np.ascontiguousarray(v.reshape(D, 1).astype(np.float32))
    f8 = ml_dtypes.float8_e4m3fn
    xbf = np.ascontiguousarray(g["x"].astype(ml_dtypes.bfloat16))
    x16t = np.ascontiguousarray(np.swapaxes((g["x"] / 16.0).astype(f8), 1, 2))
    xf8 = np.ascontiguousarray(g["x"].astype(f8))
    in_maps = []
    for c in range(NCORES):
        m = dict(base)
        m["xbf"] = np.ascontiguousarray(xbf[c * BPC:(c + 1) * BPC])
        m["x16t"] = np.ascontiguousarray(x16t[c * BPC:(c + 1) * BPC])
        m["xf8"] = np.ascontiguousarray(xf8[c * BPC:(c + 1) * BPC])
        in_maps.append(m)
    return in_maps, modes


def kernel(**inputs):
    from concourse.bass_utils import run_bass_kernel_spmd

    in_maps, modes = _prep_inputs(inputs)
    key = tuple(sorted((n, k[0], k[1]) for n, k in modes.items()))
    if _CACHE.get("key") != key:
        _CACHE["nc"] = _build_nc(modes)
        _CACHE["key"] = key
    nc = _CACHE["nc"]

    res = run_bass_kernel_spmd(nc, in_maps, list(range(NCORES)))
    out = np.concatenate([r["out"] for r in res.results], axis=0)
    return out.astype(np.float32)


# revision 20
# speedup vs baseline: 1.1177x; 1.0245x over previous
"""Trainium2 Bass kernel for nn_Net_5488968204310 (gnn_message_passing).

Single-head self-attention (D=128) over N=1024 nodes + gated residual update,
batch B=32, data-parallel across 8 NeuronCores (4 samples per core).

Design notes:
  - "T layout" (features d on partitions, nodes on free dim) for every matmul;
    contraction is always over d.
  - QK fold: logit_ij = x_i.(M^T x_j) with M = Wq@Wk^T folded on the host, so
    only ONE projection matmul (stationary S = Wk@Wq^T) is needed and the
    logits' moving operand is xT itself. The per-query bias term is
    softmax-invariant and dropped; the per-key term folds into y's bias.
  - V fold: attn@Wo = (A@X)@(Wv@Wo). Attention runs over raw X (host-cast
    fp8 upload, natural layout) and Wv folds into the tail weights, removing
    the v projection and its PSUM->SBUF cast.
  - QK^T: logitsT chunks [m_chunk(128) x q(1024)] = yT_chunk.T @ xT; exp() on
    the scalar engine straight from PSUM with the 1/sqrt(D) scale and a -2
    bias folded in (rescales exp into fp8 range; the uniform e^-2 factor
    cancels between numerator and denominator).
  - A@X keeps xf8 as the stationary operand (DoubleRow fp8, dense 512-col
    streams). The softmax denominator is a parallel ones.T @ expw DoubleRow
    accumulation; 1/denom via the DVE reciprocal_approx_fast custom op.
  - gate sigmoid as 0.5*(1+tanh(z/2)): one ACT op (Tanh) instead of a 3-op
    exp/ln chain. The 0.5 gate factor is folded into the u-path weights on
    the host so dlt = u_half * (tanh+1) is a single DVE scalar_tensor_tensor
    reading u DIRECTLY from PSUM (parked in the idle dn/av banks during the
    tail) -- no u copyback instruction at all.
  - ACT functions used (Exp, Tanh, Identity/Relu fallbacks) all live in the
    exp_and_others table set, enforced by a scoped patch of the table
    metadata at compile time: exactly one ACT_TABLE_LOAD per run.
  - x ships three ways from the host (bf16 natural, bf16 pre-transposed,
    fp8 natural) so no on-chip transposes or casts sit on the input path;
    the six folded 128x128 weights ship as ONE packed DMA.
  - A burst of dummy DoubleRow matmuls on the ones constant warms the PE out
    of its cold p-state while the first x DMAs are still in flight.
  - The residual add runs in bf16 on the Pool engine, which also issues the
    output stores so they chain straight after the add with no engine hop.
"""

import math

import numpy as np
import ml_dtypes

B, N, D = 32, 1024, 128
NCORES = 8
BPC = B // NCORES  # samples per core
NT = N // 128      # node chunks per sample

WNAMES = ["Wm", "Woh", "Wo1mh", "Wg1", "Wog2", "Wg3"]

_CACHE = {}


def _bias_mode(vec):
    """(kind, value) where kind in {'zero', 'uniform', 'ap'}."""
    v = np.asarray(vec, np.float32)
    if not np.any(v):
        return ("zero", 0.0)
    if np.all(v == v.flat[0]):
        return ("uniform", float(v.flat[0]))
    return ("ap", 0.0)


def _build_nc(modes):
    import concourse.bacc as bacc
    import concourse.tile as tile
    from concourse import mybir
    from contextlib import ExitStack

    f32 = mybir.dt.float32
    bf16 = mybir.dt.bfloat16
    f8 = mybir.dt.float8e4
    AF = mybir.ActivationFunctionType
    OP = mybir.AluOpType
    DR = mybir.MatmulPerfMode.DoubleRow

    nc = bacc.Bacc("TRN2", target_bir_lowering=False, debug=False)

    xb_d = nc.dram_tensor("xbf", [BPC, N, D], bf16, kind="ExternalInput")
    xt_d = nc.dram_tensor("x16t", [BPC, D, N], f8, kind="ExternalInput")
    xf8_d = nc.dram_tensor("xf8", [BPC, N, D], f8, kind="ExternalInput")
    out_d = nc.dram_tensor("out", [BPC, N, D], f32, kind="ExternalOutput")
    wp_d = nc.dram_tensor("wpack8", [D, 5, D], f8, kind="ExternalInput")
    wg3_d = nc.dram_tensor("wg3", [D, D], bf16, kind="ExternalInput")
    b_d = {
        n: nc.dram_tensor(n, [D, 1], f32, kind="ExternalInput")
        for n in modes if modes[n][0] == "ap"
    }

    s = 1.0 / math.sqrt(D)

    with tile.TileContext(nc) as tc, ExitStack() as ctx:
        consts = ctx.enter_context(tc.tile_pool(name="consts", bufs=1))
        sb = ctx.enter_context(tc.tile_pool(name="sb", bufs=2))
        sb3 = ctx.enter_context(tc.tile_pool(name="sb3", bufs=3))
        expp = ctx.enter_context(tc.tile_pool(name="expp", bufs=2))
        pw = ctx.enter_context(tc.tile_pool(name="pw", bufs=2, space="PSUM"))
        ph = ctx.enter_context(tc.tile_pool(name="ph", bufs=2, space="PSUM"))
        pav = ctx.enter_context(tc.tile_pool(name="pav", bufs=1, space="PSUM"))
        pden = ctx.enter_context(tc.tile_pool(name="pden", bufs=1, space="PSUM"))

        ST = {}

        def load(b):
            """input DMAs for sample b (issued one pipeline step early)."""
            st = {}
            # two-slot tile: slot 0 <- attnX/16 (written by the DVE mul in
            # phase23), slot 1 <- x/16 transposed (DMA). The fp8 DoubleRow
            # tail matmuls consume both slots in one instruction.
            axT = sb3.tile([128, 2, N], f8, tag="axT")  # [d, slot, n]
            nc.sync.dma_start(axT[:, 1, :], xt_d[b])
            x_nat = sb3.tile([128, NT, D], bf16, tag="x_nat")
            nc.sync.dma_start(x_nat, xb_d[b].rearrange("(c p) d -> p c d", p=128))
            xf8_nat = sb3.tile([128, NT, D], f8, tag="xf8_nat")
            nc.sync.dma_start(xf8_nat, xf8_d[b].rearrange("(c p) d -> p c d", p=128))
            st["x_nat"], st["xf8_nat"], st["axT"] = x_nat, xf8_nat, axT
            return st

        # input DMAs for sample 0 go out before anything else; the packed
        # weight DMA + consts follow on other engines so nothing serializes
        # behind the sequencer's per-DMA issue cost.
        ST[0] = load(0)

        wpack = consts.tile([D, 5, D], f8, tag="wpack")
        nc.scalar.dma_start(wpack, wp_d[:, :, :])
        wg3 = consts.tile([D, D], bf16, tag="wg3")
        nc.scalar.dma_start(wg3, wg3_d[:, :])
        Wm, Wu, Wg = wpack[:, 0, :], wpack[:, 1:3, :], wpack[:, 3:5, :]
        # ones = 16 so the softmax denominator absorbs the 1/16 activation
        # scale: attnT = p_av/(16*sum) pairs exactly with the 16x weights.
        ones_dr = consts.tile([128, 2, 128], f8, tag="ones_dr")
        nc.gpsimd.memset(ones_dr, 16.0)
        expbias = consts.tile([128, 1], f32, tag="expbias")
        nc.gpsimd.memset(expbias, -2.0)
        BV = {}
        for n in b_d:
            t = consts.tile([D, 1], f32, tag=f"b_{n}")
            nc.scalar.dma_start(t, b_d[n][:, :])
            BV[n] = t
        for n, (kind, val) in modes.items():
            if kind == "uniform":
                t = consts.tile([D, 1], f32, tag=f"b_{n}")
                nc.gpsimd.memset(t, val)
                BV[n] = t

        # PE p-state warmup: dummy DoubleRow matmuls on the ones const while
        # the first x DMAs are still in flight, so the first real matmuls run
        # at full clock instead of the cold 0.65 GHz p-state.
        pdum = pden.tile([128, 512], f32, tag="pden", name="pdum")
        for _ in range(16):
            nc.tensor.matmul(pdum[:, 0:128], ones_dr, ones_dr,
                             start=True, stop=True, perf_mode=DR)

        def phase1(st):
            """y projection, QK^T + exp (all fp8, 1/16-scaled operands; the
            1/256 logit scale folds into the exp activation scale)."""
            xs = st["axT"][:, 1, :]

            p_y = pw.tile([128, N], f32, tag="pw")
            nc.tensor.matmul(p_y[:, 0:512], Wm, xs[:, 0:512], start=True, stop=True)
            nc.tensor.matmul(p_y[:, 512:1024], Wm, xs[:, 512:1024], start=True, stop=True)
            yT = sb.tile([128, N], f8, tag="yT")
            if modes["by"][0] == "zero":
                nc.vector.tensor_copy(yT[:, 0:512], p_y[:, 0:512])
                nc.vector.tensor_copy(yT[:, 512:1024], p_y[:, 512:1024])
            else:
                nc.scalar.activation(yT[:, 0:512], p_y[:, 0:512], AF.Identity, bias=BV["by"])
                nc.scalar.activation(yT[:, 512:1024], p_y[:, 512:1024], AF.Identity, bias=BV["by"])

            expw = expp.tile([128, NT, N], f8, tag="expw")  # [m', c_m, q]
            for c in range(NT):
                p_l = pw.tile([128, N], f32, tag="pw")
                yTc = yT[:, c * 128:(c + 1) * 128]
                nc.tensor.matmul(p_l[:, 0:512], yTc, xs[:, 0:512], start=True, stop=True)
                nc.tensor.matmul(p_l[:, 512:1024], yTc, xs[:, 512:1024], start=True, stop=True)
                nc.scalar.activation(expw[:, c, :], p_l, AF.Exp, scale=s * 256.0, bias=expbias)
            st["expw"] = expw

        def phase23(b, st):
            """softmax normalize + gated update tail; store.

            u = 0.5*(ret - x) stays in PSUM (parked in the dn/av banks) and
            is consumed directly by the dlt scalar_tensor_tensor;
            gate = 0.5*(1 + tanh((z+bg3)/2)); out = x + u*(1+tanh)."""
            expw, xf8_nat = st["expw"], st["xf8_nat"]
            x_nat, axT = st["x_nat"], st["axT"]
            rb = sb.tile([128, N], f32, tag="rb")
            gp = sb.tile([128, N], bf16, tag="gp")
            last = b == BPC - 1
            if last:
                # keep the PE clock ramped across the last-exp boundary so the
                # drain's dn/av run at full speed (dummies depend on nothing
                # and retire while the final exps drain).
                for _ in range(4):
                    nc.tensor.matmul(pdum[:, 0:128], ones_dr, ones_dr,
                                     start=True, stop=True, perf_mode=DR)
            th = sb.tile([128, N], bf16, tag="th")
            dlt = sb.tile([128, N], bf16, tag="dlt")
            dlt_nat = sb.tile([128, NT, 128], bf16, tag="dlt_nat")
            o = sb.tile([128, NT, D], f32, tag="o")
            out_r = out_d[b].rearrange("(c p) d -> p c d", p=128)
            H = NT // 2
            tanh_bias = BV["bg3h"] if "bg3h" in BV else 0.0

            for h in range(2):
                sl = slice(h * 512, (h + 1) * 512)
                p_dn = pden.tile([128, 512], f32, tag="pden")
                for c in range(NT // 2):
                    nc.tensor.matmul(p_dn, ones_dr, expw[:, 2 * c:2 * c + 2, sl],
                                     start=(c == 0), stop=(c == NT // 2 - 1), perf_mode=DR)
                nc.vector.reciprocal_approx_fast(rb[:, sl], p_dn)
                p_av = pav.tile([128, 512], f32, tag="pav")
                for c in range(NT // 2):
                    nc.tensor.matmul(p_av, xf8_nat[:, 2 * c:2 * c + 2, :],
                                     expw[:, 2 * c:2 * c + 2, sl],
                                     start=(c == 0), stop=(c == NT // 2 - 1), perf_mode=DR)
                nc.vector.tensor_mul(axT[:, 0, sl], p_av, rb[:, sl])

            upool = [pden, pav]
            utags = ["pden", "pav"]
            for h in range(2):
                sl = slice(h * 512, (h + 1) * 512)
                cs = slice(h * H, (h + 1) * H)

                p_m = upool[h].tile([128, 512], f32, tag=utags[h], name="p_m")
                nc.tensor.matmul(p_m, Wu, axT[:, :, sl], start=True, stop=True, perf_mode=DR)
                if modes["bo_uh"][0] != "zero":
                    nc.scalar.activation(p_m, p_m, AF.Identity, bias=BV["bo_uh"])

                p_g = ph.tile([128, 512], f32, tag="pwh")
                nc.tensor.matmul(p_g, Wg, axT[:, :, sl], start=True, stop=True, perf_mode=DR)
                if modes["bo_g"][0] == "zero":
                    nc.vector.tensor_scalar(gp[:, sl], p_g, 0.0, None, op0=OP.max)
                else:
                    nc.scalar.activation(gp[:, sl], p_g, AF.Relu, bias=BV["bo_g"])

                p_g3 = ph.tile([128, 512], f32, tag="pwh")
                nc.tensor.matmul(p_g3, wg3, gp[:, sl], start=True, stop=True)
                nc.scalar.activation(th[:, sl], p_g3, AF.Tanh, scale=0.5, bias=tanh_bias)
                nc.vector.scalar_tensor_tensor(
                    dlt[:, sl], th[:, sl], 1.0, p_m, op0=OP.add, op1=OP.mult
                )
                if not last:
                    nc.sync.dma_start_transpose(dlt_nat[:, cs, :], dlt[:, sl])
                    nc.gpsimd.tensor_add(o[:, cs, :], dlt_nat[:, cs, :], x_nat[:, cs, :])
                    nc.gpsimd.dma_start(out_r[:, cs, :], o[:, cs, :])
                else:
                    # drain fast-path: quarter-granularity transpose -> add ->
                    # store pipeline on alternating engines so the xbar, the
                    # adders and the store queues all overlap.
                    for q in range(2):
                        cq = slice(h * H + q * 2, h * H + (q + 1) * 2)
                        sq = slice(h * 512 + q * 256, h * 512 + (q + 1) * 256)
                        teng = [nc.sync, nc.scalar][q]
                        teng.dma_start_transpose(dlt_nat[:, cq, :], dlt[:, sq])
                        aeng = [nc.vector, nc.gpsimd][q]
                        aeng.tensor_add(o[:, cq, :], dlt_nat[:, cq, :], x_nat[:, cq, :])
                        seng = [nc.sync, nc.gpsimd][q]
                        seng.dma_start(out_r[:, cq, :], o[:, cq, :])

        # Software pipeline: emit P1(k-1), P23(k-2), Load(k) per step so each
        # engine's in-order stream interleaves two samples and input DMAs run
        # a full step ahead of first use.
        for k in range(1, BPC + 2):
            if 0 <= k - 1 < BPC:
                phase1(ST[k - 1])
            if 0 <= k - 2:
                phase23(k - 2, ST[k - 2])
            if k < BPC:
                ST[k] = load(k)

    # Force Exp and Tanh to resolve to the one table set that holds both
    # (exp_and_others): contents-only lie to the set chooser, dict order
    # (= act_func_set_id) preserved; the set actually loaded at runtime does
    # contain both functions (plus Identity/Relu used by bias fallbacks).
    import concourse.bacc as bacc_mod

    real_get = bacc_mod.get_activation_tables
    target = "exp_and_others"

    def patched_get(arch):
        tabs = real_get(arch)
        strip = {AF.Exp, AF.Tanh}
        return {
            name: (set(fns) if name == target else set(fns) - strip)
            for name, fns in tabs.items()
        }

    bacc_mod.get_activation_tables = patched_get
    try:
        nc.compile()
    finally:
        bacc_mod.get_activation_tables = real_get
    return nc


def _prep_host(inputs):
    """Host-side: fold weights/biases; returns (f32 inputs, wpack bf16, biases)."""
    f32 = np.float32
    g = {k: np.asarray(v, f32) for k, v in inputs.items()}

    Wm = g["Wk"] @ g["Wq"].T                       # y = x@Wk@Wq^T; logit=x_i.y_j
    Wvo = g["Wv"] @ g["Wo"]                        # v path folded into tail
    Wo1m = g["Wo1"] - np.eye(D, dtype=f32)
    Wog2 = Wvo @ g["Wg2"]                          # msg path folded into gate
    bo_msg = g["bo"] + g["bv"] @ g["Wo"]           # bv folded through Wo
    bo_uh = 0.5 * (bo_msg + g["bo1"])              # msg bias + ret bias, halved
    bo_g = bo_msg @ g["Wg2"] + g["bg1"] + g["bg2"]
    bg3h = 0.5 * g["bg3"]                          # tanh((z+bg3)/2) bias
    by = g["Wk"] @ g["bq"]                         # per-key logit bias

    # fp8 pack: [Wm, 16*Woh, 16*Wo1mh, 16*Wog2, 16*Wg1]; the 16x weight
    # scale cancels against the 1/16-scaled x/attn activations exactly.
    f8 = ml_dtypes.float8_e4m3fn
    wpack8 = np.stack(
        [Wm, 8.0 * Wvo, 8.0 * Wo1m, 16.0 * Wog2, 16.0 * g["Wg1"]], axis=1
    ).astype(f8)
    wg3 = g["Wg3"].astype(ml_dtypes.bfloat16)
    bmap = {
        "by": by / 16.0,
        "bo_uh": bo_uh, "bo_g": bo_g, "bg3h": bg3h,
    }
    return g, (np.ascontiguousarray(wpack8), np.ascontiguousarray(wg3)), bmap


def _prep_inputs(inputs):
    g, (wpack8, wg3), bmap = _prep_host(inputs)
    modes = {n: _bias_mode(v) for n, v in bmap.items()}
    base = {"wpack8": wpack8, "wg3": wg3}
    for n, v in bmap.items():
        if modes[n][0] == "ap":
            base[n] = np.ascontiguousarray(v.reshape(D, 1).astype(np.float32))
    f8 = ml_dtypes.float8_e4m3fn
    xbf = np.ascontiguousarray(g["x"].astype(ml_dtypes.bfloat16))
    x16t = np.ascontiguousarray(np.swapaxes((g["x"] / 16.0).astype(f8), 1, 2))
    xf8 = np.ascontiguousarray(g["x"].astype(f8))
    in_maps = []
    for c in range(NCORES):
        m = dict(base)
        m["xbf"] = np.ascontiguousarray(xbf[c * BPC:(c + 1) * BPC])
        m["x16t"] = np.ascontiguousarray(x16t[c * BPC:(c + 1) * BPC])
        m["xf8"] = np.ascontiguousarray(xf8[c * BPC:(c + 1) * BPC])
        in_maps.append(m)
    return in_maps, modes


def kernel(**inputs):
    from concourse.bass_utils import run_bass_kernel_spmd

    in_maps, modes = _prep_inputs(inputs)
    key = tuple(sorted((n, k[0], k[1]) for n, k in modes.items()))
    if _CACHE.get("key") != key:
        _CACHE["nc"] = _build_nc(modes)
        _CACHE["key"] = key
    nc = _CACHE["nc"]

    res = run_bass_kernel_spmd(nc, in_maps, list(range(NCORES)))
    out = np.concatenate([r["out"] for r in res.results], axis=0)
    return out.astype(np.float32)


# revision 22
# speedup vs baseline: 1.1382x; 1.0183x over previous
"""Trainium2 Bass kernel for nn_Net_5488968204310 (gnn_message_passing).

Single-head self-attention (D=128) over N=1024 nodes + gated residual update,
batch B=32, data-parallel across 8 NeuronCores (4 samples per core).

Design notes:
  - "T layout" (features d on partitions, nodes on free dim) for every matmul;
    contraction is always over d.
  - QK fold: logit_ij = x_i.(M^T x_j) with M = Wq@Wk^T folded on the host, so
    only ONE projection matmul (stationary S = Wk@Wq^T) is needed and the
    logits' moving operand is xT itself. The per-query bias term is
    softmax-invariant and dropped; the per-key term folds into y's bias.
  - V fold: attn@Wo = (A@X)@(Wv@Wo). Attention runs over raw X (host-cast
    fp8 upload, natural layout) and Wv folds into the tail weights, removing
    the v projection and its PSUM->SBUF cast.
  - QK^T: logitsT chunks [m_chunk(128) x q(1024)] = yT_chunk.T @ xT; exp() on
    the scalar engine straight from PSUM with the 1/sqrt(D) scale and a -2
    bias folded in (rescales exp into fp8 range; the uniform e^-2 factor
    cancels between numerator and denominator).
  - A@X keeps xf8 as the stationary operand (DoubleRow fp8, dense 512-col
    streams). The softmax denominator is a parallel ones.T @ expw DoubleRow
    accumulation; 1/denom via the DVE reciprocal_approx_fast custom op.
  - gate sigmoid as 0.5*(1+tanh(z/2)): one ACT op (Tanh) instead of a 3-op
    exp/ln chain. The 0.5 gate factor is folded into the u-path weights on
    the host so dlt = u_half * (tanh+1) is a single DVE scalar_tensor_tensor
    reading u DIRECTLY from PSUM (parked in the idle dn/av banks during the
    tail) -- no u copyback instruction at all.
  - ACT functions used (Exp, Tanh, Identity/Relu fallbacks) all live in the
    exp_and_others table set, enforced by a scoped patch of the table
    metadata at compile time: exactly one ACT_TABLE_LOAD per run.
  - x ships three ways from the host (bf16 natural, bf16 pre-transposed,
    fp8 natural) so no on-chip transposes or casts sit on the input path;
    the six folded 128x128 weights ship as ONE packed DMA.
  - A burst of dummy DoubleRow matmuls on the ones constant warms the PE out
    of its cold p-state while the first x DMAs are still in flight.
  - The residual add runs in bf16 on the Pool engine, which also issues the
    output stores so they chain straight after the add with no engine hop.
"""

import math

import numpy as np
import ml_dtypes

B, N, D = 32, 1024, 128
NCORES = 8
BPC = B // NCORES  # samples per core
NT = N // 128      # node chunks per sample

WNAMES = ["Wm", "Woh", "Wo1mh", "Wg1", "Wog2", "Wg3"]

_CACHE = {}


def _bias_mode(vec):
    """(kind, value) where kind in {'zero', 'uniform', 'ap'}."""
    v = np.asarray(vec, np.float32)
    if not np.any(v):
        return ("zero", 0.0)
    if np.all(v == v.flat[0]):
        return ("uniform", float(v.flat[0]))
    return ("ap", 0.0)


def _build_nc(modes):
    import concourse.bacc as bacc
    import concourse.tile as tile
    from concourse import mybir
    from contextlib import ExitStack

    f32 = mybir.dt.float32
    bf16 = mybir.dt.bfloat16
    f8 = mybir.dt.float8e4
    AF = mybir.ActivationFunctionType
    OP = mybir.AluOpType
    DR = mybir.MatmulPerfMode.DoubleRow

    nc = bacc.Bacc("TRN2", target_bir_lowering=False, debug=False)

    xb_d = nc.dram_tensor("xbf", [BPC, N, D], bf16, kind="ExternalInput")
    xt_d = nc.dram_tensor("x16t", [BPC, D, N], f8, kind="ExternalInput")
    xf8_d = nc.dram_tensor("xf8", [BPC, N, D], f8, kind="ExternalInput")
    out_d = nc.dram_tensor("out", [BPC, N, D], f32, kind="ExternalOutput")
    wp_d = nc.dram_tensor("wpack8", [D, 5, D], f8, kind="ExternalInput")
    wg3_d = nc.dram_tensor("wg3", [D, D], bf16, kind="ExternalInput")
    b_d = {
        n: nc.dram_tensor(n, [D, 1], f32, kind="ExternalInput")
        for n in modes if modes[n][0] == "ap"
    }

    s = 1.0 / math.sqrt(D)

    with tile.TileContext(nc) as tc, ExitStack() as ctx:
        consts = ctx.enter_context(tc.tile_pool(name="consts", bufs=1))
        sb = ctx.enter_context(tc.tile_pool(name="sb", bufs=2))
        sb3 = ctx.enter_context(tc.tile_pool(name="sb3", bufs=3))
        expp = ctx.enter_context(tc.tile_pool(name="expp", bufs=2))
        pw = ctx.enter_context(tc.tile_pool(name="pw", bufs=2, space="PSUM"))
        ph = ctx.enter_context(tc.tile_pool(name="ph", bufs=2, space="PSUM"))
        pav = ctx.enter_context(tc.tile_pool(name="pav", bufs=1, space="PSUM"))
        pden = ctx.enter_context(tc.tile_pool(name="pden", bufs=1, space="PSUM"))

        ST = {}

        def load(b):
            """input DMAs for sample b (issued one pipeline step early)."""
            st = {}
            # two-slot tile: slot 0 <- attnX/16 (written by the DVE mul in
            # phase23), slot 1 <- x/16 transposed (DMA). The fp8 DoubleRow
            # tail matmuls consume both slots in one instruction.
            axT = sb3.tile([128, 2, N], f8, tag="axT")  # [d, slot, n]
            nc.sync.dma_start(axT[:, 1, :], xt_d[b])
            x_nat = sb3.tile([128, NT, D], bf16, tag="x_nat")
            nc.sync.dma_start(x_nat, xb_d[b].rearrange("(c p) d -> p c d", p=128))
            xf8_nat = sb3.tile([128, NT, D], f8, tag="xf8_nat")
            nc.sync.dma_start(xf8_nat, xf8_d[b].rearrange("(c p) d -> p c d", p=128))
            st["x_nat"], st["xf8_nat"], st["axT"] = x_nat, xf8_nat, axT
            return st

        # input DMAs for sample 0 go out before anything else; the packed
        # weight DMA + consts follow on other engines so nothing serializes
        # behind the sequencer's per-DMA issue cost.
        ST[0] = load(0)

        wpack = consts.tile([D, 5, D], f8, tag="wpack")
        nc.scalar.dma_start(wpack, wp_d[:, :, :])
        wg3 = consts.tile([D, D], bf16, tag="wg3")
        nc.scalar.dma_start(wg3, wg3_d[:, :])
        Wm, Wu, Wg = wpack[:, 0, :], wpack[:, 1:3, :], wpack[:, 3:5, :]
        # ones = 16 so the softmax denominator absorbs the 1/16 activation
        # scale: attnT = p_av/(16*sum) pairs exactly with the 16x weights.
        ones_dr = consts.tile([128, 2, 128], f8, tag="ones_dr")
        nc.gpsimd.memset(ones_dr, 16.0)
        expbias = consts.tile([128, 1], f32, tag="expbias")
        nc.gpsimd.memset(expbias, -2.0)
        BV = {}
        for n in b_d:
            t = consts.tile([D, 1], f32, tag=f"b_{n}")
            nc.scalar.dma_start(t, b_d[n][:, :])
            BV[n] = t
        for n, (kind, val) in modes.items():
            if kind == "uniform":
                t = consts.tile([D, 1], f32, tag=f"b_{n}")
                nc.gpsimd.memset(t, val)
                BV[n] = t

        # first-instruction warmup for ACT/DVE: tiny dep-free ops so the
        # engines' sequencer/sem machinery is primed before the real chain.
        wsc = consts.tile([128, 1], f32, tag="wsc")
        nc.vector.memset(wsc, 1.0)
        nc.scalar.activation(wsc, expbias, AF.Exp)

        # PE p-state warmup: dummy DoubleRow matmuls on the ones const while
        # the first x DMAs are still in flight, so the first real matmuls run
        # at full clock instead of the cold 0.65 GHz p-state.
        pdum = pden.tile([128, 512], f32, tag="pden", name="pdum")
        for _ in range(16):
            nc.tensor.matmul(pdum[:, 0:128], ones_dr, ones_dr,
                             start=True, stop=True, perf_mode=DR)

        def prefetch_y(st):
            """y projection (emitted one step early so each exp batch starts
            with its stationary operand already in SBUF)."""
            xs = st["axT"][:, 1, :]
            p_y = pw.tile([128, N], f32, tag="pw")
            nc.tensor.matmul(p_y[:, 0:512], Wm, xs[:, 0:512], start=True, stop=True)
            nc.tensor.matmul(p_y[:, 512:1024], Wm, xs[:, 512:1024], start=True, stop=True)
            yT = sb.tile([128, N], f8, tag="yT")
            if modes["by"][0] == "zero":
                nc.vector.tensor_copy(yT[:, 0:512], p_y[:, 0:512])
                nc.vector.tensor_copy(yT[:, 512:1024], p_y[:, 512:1024])
            else:
                nc.scalar.activation(yT[:, 0:512], p_y[:, 0:512], AF.Identity, bias=BV["by"])
                nc.scalar.activation(yT[:, 512:1024], p_y[:, 512:1024], AF.Identity, bias=BV["by"])
            st["yT"] = yT

        def phase1(st):
            """QK^T + exp (all fp8, 1/16-scaled operands; the 1/256 logit
            scale folds into the exp activation scale)."""
            xs = st["axT"][:, 1, :]
            yT = st["yT"]
            expw = expp.tile([128, NT, N], f8, tag="expw")  # [m', c_m, q]
            for c in range(NT):
                p_l = pw.tile([128, N], f32, tag="pw")
                yTc = yT[:, c * 128:(c + 1) * 128]
                nc.tensor.matmul(p_l[:, 0:512], yTc, xs[:, 0:512], start=True, stop=True)
                nc.tensor.matmul(p_l[:, 512:1024], yTc, xs[:, 512:1024], start=True, stop=True)
                nc.scalar.activation(expw[:, c, :], p_l, AF.Exp, scale=s * 256.0, bias=expbias)
            st["expw"] = expw

        def phase23(b, st):
            """softmax normalize + gated update tail; store.

            u = 0.5*(ret - x) stays in PSUM (parked in the dn/av banks) and
            is consumed directly by the dlt scalar_tensor_tensor;
            gate = 0.5*(1 + tanh((z+bg3)/2)); out = x + u*(1+tanh)."""
            expw, xf8_nat = st["expw"], st["xf8_nat"]
            x_nat, axT = st["x_nat"], st["axT"]
            rb = sb.tile([128, N], f32, tag="rb")
            gp = sb.tile([128, N], bf16, tag="gp")
            last = b == BPC - 1
            if last:
                # keep the PE clock ramped across the last-exp boundary so the
                # drain's dn/av run at full speed: the bridge matmuls read
                # only the batch's EARLY exp chunks, so they retire while the
                # final exps drain instead of stalling behind them.
                pdw = pw.tile([128, N], f32, tag="pw", name="pdw")
                for r in range(8):
                    nc.tensor.matmul(pdw[:, 0:512], ones_dr, expw[:, 0:2, 0:512],
                                     start=True, stop=True, perf_mode=DR)
            th = sb.tile([128, N], bf16, tag="th")
            dlt = sb.tile([128, N], bf16, tag="dlt")
            dlt_nat = sb.tile([128, NT, 128], bf16, tag="dlt_nat")
            o = sb.tile([128, NT, D], f32, tag="o")
            out_r = out_d[b].rearrange("(c p) d -> p c d", p=128)
            H = NT // 2
            tanh_bias = BV["bg3h"] if "bg3h" in BV else 0.0

            for h in range(2):
                sl = slice(h * 512, (h + 1) * 512)
                p_dn = pden.tile([128, 512], f32, tag="pden")
                for c in range(NT // 2):
                    nc.tensor.matmul(p_dn, ones_dr, expw[:, 2 * c:2 * c + 2, sl],
                                     start=(c == 0), stop=(c == NT // 2 - 1), perf_mode=DR)
                nc.vector.reciprocal_approx_fast(rb[:, sl], p_dn)
                p_av = pav.tile([128, 512], f32, tag="pav")
                for c in range(NT // 2):
                    nc.tensor.matmul(p_av, xf8_nat[:, 2 * c:2 * c + 2, :],
                                     expw[:, 2 * c:2 * c + 2, sl],
                                     start=(c == 0), stop=(c == NT // 2 - 1), perf_mode=DR)
                nc.vector.tensor_mul(axT[:, 0, sl], p_av, rb[:, sl])

            upool = [pden, pav]
            utags = ["pden", "pav"]
            for h in range(2):
                sl = slice(h * 512, (h + 1) * 512)
                cs = slice(h * H, (h + 1) * H)

                p_m = upool[h].tile([128, 512], f32, tag=utags[h], name="p_m")
                nc.tensor.matmul(p_m, Wu, axT[:, :, sl], start=True, stop=True, perf_mode=DR)
                if modes["bo_uh"][0] != "zero":
                    nc.scalar.activation(p_m, p_m, AF.Identity, bias=BV["bo_uh"])

                p_g = ph.tile([128, 512], f32, tag="pwh")
                nc.tensor.matmul(p_g, Wg, axT[:, :, sl], start=True, stop=True, perf_mode=DR)
                if modes["bo_g"][0] == "zero":
                    nc.vector.tensor_scalar(gp[:, sl], p_g, 0.0, None, op0=OP.max)
                else:
                    nc.scalar.activation(gp[:, sl], p_g, AF.Relu, bias=BV["bo_g"])

                p_g3 = ph.tile([128, 512], f32, tag="pwh")
                nc.tensor.matmul(p_g3, wg3, gp[:, sl], start=True, stop=True)
                nc.scalar.activation(th[:, sl], p_g3, AF.Tanh, scale=0.5, bias=tanh_bias)
                nc.vector.scalar_tensor_tensor(
                    dlt[:, sl], th[:, sl], 1.0, p_m, op0=OP.add, op1=OP.mult
                )
                if not last:
                    nc.sync.dma_start_transpose(dlt_nat[:, cs, :], dlt[:, sl])
                    nc.gpsimd.tensor_add(o[:, cs, :], dlt_nat[:, cs, :], x_nat[:, cs, :])
                    nc.gpsimd.dma_start(out_r[:, cs, :], o[:, cs, :])
                else:
                    # drain fast-path: quarter-granularity transpose -> add ->
                    # store pipeline on alternating engines so the xbar, the
                    # adders and the store queues all overlap.
                    for q in range(2):
                        cq = slice(h * H + q * 2, h * H + (q + 1) * 2)
                        sq = slice(h * 512 + q * 256, h * 512 + (q + 1) * 256)
                        teng = [nc.sync, nc.scalar][q]
                        teng.dma_start_transpose(dlt_nat[:, cq, :], dlt[:, sq])
                        aeng = [nc.vector, nc.gpsimd][q]
                        aeng.tensor_add(o[:, cq, :], dlt_nat[:, cq, :], x_nat[:, cq, :])
                        seng = [nc.sync, nc.gpsimd][q]
                        seng.dma_start(out_r[:, cq, :], o[:, cq, :])

        # Software pipeline: emit P1(k-1), P23(k-2), Load(k) per step so each
        # engine's in-order stream interleaves two samples and input DMAs run
        # a full step ahead of first use.
        prefetch_y(ST[0])
        for k in range(1, BPC + 2):
            if 0 <= k - 1 < BPC:
                phase1(ST[k - 1])
            if k < BPC:
                ST[k] = load(k)
            if 0 <= k - 2:
                phase23(k - 2, ST[k - 2])
            if k < BPC:
                prefetch_y(ST[k])

    # Force Exp and Tanh to resolve to the one table set that holds both
    # (exp_and_others): contents-only lie to the set chooser, dict order
    # (= act_func_set_id) preserved; the set actually loaded at runtime does
    # contain both functions (plus Identity/Relu used by bias fallbacks).
    import concourse.bacc as bacc_mod

    real_get = bacc_mod.get_activation_tables
    target = "exp_and_others"

    def patched_get(arch):
        tabs = real_get(arch)
        strip = {AF.Exp, AF.Tanh}
        return {
            name: (set(fns) if name == target else set(fns) - strip)
            for name, fns in tabs.items()
        }

    bacc_mod.get_activation_tables = patched_get
    try:
        nc.compile()
    finally:
        bacc_mod.get_activation_tables = real_get
    return nc


def _prep_host(inputs):
    """Host-side: fold weights/biases; returns (f32 inputs, wpack bf16, biases)."""
    f32 = np.float32
    g = {k: np.asarray(v, f32) for k, v in inputs.items()}

    Wm = g["Wk"] @ g["Wq"].T                       # y = x@Wk@Wq^T; logit=x_i.y_j
    Wvo = g["Wv"] @ g["Wo"]                        # v path folded into tail
    Wo1m = g["Wo1"] - np.eye(D, dtype=f32)
    Wog2 = Wvo @ g["Wg2"]                          # msg path folded into gate
    bo_msg = g["bo"] + g["bv"] @ g["Wo"]           # bv folded through Wo
    bo_uh = 0.5 * (bo_msg + g["bo1"])              # msg bias + ret bias, halved
    bo_g = bo_msg @ g["Wg2"] + g["bg1"] + g["bg2"]
    bg3h = 0.5 * g["bg3"]                          # tanh((z+bg3)/2) bias
    by = g["Wk"] @ g["bq"]                         # per-key logit bias

    # fp8 pack: [Wm, 16*Woh, 16*Wo1mh, 16*Wog2, 16*Wg1]; the 16x weight
    # scale cancels against the 1/16-scaled x/attn activations exactly.
    f8 = ml_dtypes.float8_e4m3fn
    wpack8 = np.stack(
        [Wm, 8.0 * Wvo, 8.0 * Wo1m, 16.0 * Wog2, 16.0 * g["Wg1"]], axis=1
    ).astype(f8)
    wg3 = g["Wg3"].astype(ml_dtypes.bfloat16)
    bmap = {
        "by": by / 16.0,
        "bo_uh": bo_uh, "bo_g": bo_g, "bg3h": bg3h,
    }
    return g, (np.ascontiguousarray(wpack8), np.ascontiguousarray(wg3)), bmap


def _prep_inputs(inputs):
    g, (wpack8, wg3), bmap = _prep_host(inputs)
    modes = {n: _bias_mode(v) for n, v in bmap.items()}
    base = {"wpack8": wpack8, "wg3": wg3}
    for n, v in bmap.items():
        if modes[n][0] == "ap":
            base[n] = np.ascontiguousarray(v.reshape(D, 1).astype(np.float32))
    f8 = ml_dtypes.float8_e4m3fn
    xbf = np.ascontiguousarray(g["x"].astype(ml_dtypes.bfloat16))
    x16t = np.ascontiguousarray(np.swapaxes((g["x"] / 16.0).astype(f8), 1, 2))
    xf8 = np.ascontiguousarray(g["x"].astype(f8))
    in_maps = []
    for c in range(NCORES):
        m = dict(base)
        m["xbf"] = np.ascontiguousarray(xbf[c * BPC:(c + 1) * BPC])
        m["x16t"] = np.ascontiguousarray(x16t[c * BPC:(c + 1) * BPC])
        m["xf8"] = np.ascontiguousarray(xf8[c * BPC:(c + 1) * BPC])
        in_maps.append(m)
    return in_maps, modes


def kernel(**inputs):
    from concourse.bass_utils import run_bass_kernel_spmd

    in_maps, modes = _prep_inputs(inputs)
    key = tuple(sorted((n, k[0], k[1]) for n, k in modes.items()))
    if _CACHE.get("key") != key:
        _CACHE["nc"] = _build_nc(modes)
        _CACHE["key"] = key
    nc = _CACHE["nc"]

    res = run_bass_kernel_spmd(nc, in_maps, list(range(NCORES)))
    out = np.concatenate([r["out"] for r in res.results], axis=0)
    return out.astype(np.float32)
